# revision 4
# baseline (speedup 1.0000x reference)
"""GAT message-passing kernel for Trainium2 (8 NeuronCores, data-parallel over batch).

Math (per batch element b, derived from the reference nn.Module):
    x        = nodes.reshape(N, D)
    self_e   = mlp2(x, self_*)            # [N, H]
    nb_e     = mlp2(x, nb_*)              # [N, H]
    U        = self_e @ comb_w1[:H]       # [N, H]   (i side)
    V        = nb_e @ comb_w1[H:] + comb_b1  # [N, H] (j side)
    scores(i,j) = leaky(U_i + V_j) @ comb_w2 + comb_b2
                = 0.8 * relu(U_i + V_j) @ w2 + 0.2*(sU_i + sV_j) + const_i
    softmax over j is invariant to per-i constants, so only
      s'(i,j) = 0.8 * relu(U_i + V_j) @ w2 + 0.2 * sV_j  matters.
    E^T[j,i] = edges[j,i] * (j != i) * exp(0.2*sV_j) * exp(0.8*relu(U_i+V_j)@w2)
    denom[i] = sum_j E^T[j,i];  gate = denom > eps;  recip = gate/denom
    out[i]   = gate * (recip * (E^T)^T @ nb_e + self_e)

Device mapping (one core per batch element):
  - Transposed "h-on-partitions" layout: partitions = (g, h) with g = i parity,
    so each DVE/ACT op builds relu(V + U_i) for TWO i's at once: [128, 512].
  - PE reduces over (g,h) with a slotted block-diagonal w2 lhsT (M=32), 16
    accumulating matmuls per 32-partition column group -> scores [128i, 512j]
    stacked in one PSUM bank.
  - ACT applies exp straight out of PSUM (bf16 out), PE transposes 128x128
    chunks, DVE multiplies by the (mask * exp(0.2 sV)) tiles -> E^T.
  - PE: ones-matmul row-reduce for denom, E^T @ nb_e for aggregation.
"""

import os
import sys

sys.path.insert(0, "/opt/trn_rl_repo")

import numpy as np
import ml_dtypes

import concourse.bass as bass
import concourse.bacc as bacc
import concourse.tile as tile
from concourse import mybir
from concourse.bass_utils import run_bass_kernel_spmd

B, N, H, D = 8, 512, 64, 128
NCORES = 8
NT = N // 128          # 4 i/j tiles of 128
NPAIR = N // 2         # 256 i-pairs
SLOTS = 32             # i-pairs per 64-partition column group
F32 = mybir.dt.float32
BF16 = mybir.dt.bfloat16
I32 = mybir.dt.int32

# Build-engine schedule for the 256 relu'd tiles: 'v' = VectorE, 'a' = ScalarE,
# 'p' = gpsimd/Pool. Tuned from profiles.
BUILD_PATTERN = os.environ.get("GAT_BUILD_PATTERN", "vva")

_CACHE = {}


def _build_module():
    nc = bacc.Bacc("TRN2", target_bir_lowering=False, debug=False, num_devices=NCORES)

    # ---- per-core data ----
    nodes = nc.dram_tensor("nodes", [N, D], F32, kind="ExternalInput")
    edges = nc.dram_tensor("edges", [N, N], I32, kind="ExternalInput")
    # ---- parameters / host-prepared constants (same on all cores) ----
    w1_self = nc.dram_tensor("w1_self", [D, H], F32, kind="ExternalInput")
    w2_self = nc.dram_tensor("w2_self", [H, H], F32, kind="ExternalInput")
    w1_nb = nc.dram_tensor("w1_nb", [D, H], F32, kind="ExternalInput")
    w2_nb = nc.dram_tensor("w2_nb", [H, H], F32, kind="ExternalInput")
    w1_cs = nc.dram_tensor("w1_cs", [H, H], F32, kind="ExternalInput")
    w1_cn = nc.dram_tensor("w1_cn", [H, H], F32, kind="ExternalInput")
    w2_c = nc.dram_tensor("w2_c", [H, 1], F32, kind="ExternalInput")
    b1_self = nc.dram_tensor("b1_self", [H, 1], F32, kind="ExternalInput")
    b1_nb = nc.dram_tensor("b1_nb", [H, 1], F32, kind="ExternalInput")
    b2_self_c = nc.dram_tensor("b2_self_c", [H, 1], F32, kind="ExternalInput")
    b2_nb_c = nc.dram_tensor("b2_nb_c", [H, 1], F32, kind="ExternalInput")
    b2_self_r = nc.dram_tensor("b2_self_r", [1, H], F32, kind="ExternalInput")
    b2_nb_r = nc.dram_tensor("b2_nb_r", [1, H], F32, kind="ExternalInput")
    b1_c = nc.dram_tensor("b1_c", [H, 1], F32, kind="ExternalInput")
    id_f32 = nc.dram_tensor("id_f32", [128, 128], F32, kind="ExternalInput")
    id_bf16 = nc.dram_tensor("id_bf16", [128, 128], BF16, kind="ExternalInput")
    w2bd = nc.dram_tensor("w2bd", [SLOTS, 128, 64], BF16, kind="ExternalInput")
    ones_r = nc.dram_tensor("ones_r", [1, 128], F32, kind="ExternalInput")
    ones_c = nc.dram_tensor("ones_c", [128, 1], BF16, kind="ExternalInput")
    notdiag = nc.dram_tensor("notdiag", [NT, 128, N], BF16, kind="ExternalInput")

    out = nc.dram_tensor("out", [N, H], F32, kind="ExternalOutput")

    scr_den = nc.dram_tensor("scr_den", [N], F32)
    scr_sv = nc.dram_tensor("scr_sv", [N], F32)

    with tile.TileContext(nc) as tc:
        _emit(nc, tc, locals())
    nc.compile()
    return nc


def _emit(nc, tc, t):
    AF = mybir.ActivationFunctionType
    OP = mybir.AluOpType

    with (
        tc.tile_pool(name="persist", bufs=1) as P,
        tc.tile_pool(name="xwork", bufs=2) as XW,
        tc.tile_pool(name="ework", bufs=2) as EW,
        tc.tile_pool(name="relu", bufs=10) as RL,
        tc.tile_pool(name="xexp", bufs=3) as XE,
        tc.tile_pool(name="small", bufs=4) as SM,
        tc.tile_pool(name="psumR", bufs=2, space="PSUM") as PR,
        tc.tile_pool(name="psumT", bufs=3, space="PSUM") as PT,
        tc.tile_pool(name="psumM", bufs=1, space="PSUM") as PM,
    ):
        # ---------- load constants ----------
        def load(name, shape, dtype, src=None):
            tl = P.tile(shape, dtype, tag=name)
            nc.sync.dma_start(out=tl[:], in_=(src if src is not None else t[name]).ap())
            return tl

        w1s = load("w1_self", [D, H], F32)
        w2s = load("w2_self", [H, H], F32)
        w1n = load("w1_nb", [D, H], F32)
        w2n = load("w2_nb", [H, H], F32)
        w1cs = load("w1_cs", [H, H], F32)
        w1cn = load("w1_cn", [H, H], F32)
        w2c = load("w2_c", [H, 1], F32)
        b1s = load("b1_self", [H, 1], F32)
        b1n = load("b1_nb", [H, 1], F32)
        b2sc = load("b2_self_c", [H, 1], F32)
        b2nc = load("b2_nb_c", [H, 1], F32)
        b2sr = load("b2_self_r", [1, H], F32)
        b2nr = load("b2_nb_r", [1, H], F32)
        b1c = load("b1_c", [H, 1], F32)
        idf = load("id_f32", [128, 128], F32)
        idb = load("id_bf16", [128, 128], BF16)
        onesr = load("ones_r", [1, 128], F32)
        onesc = load("ones_c", [128, 1], BF16)
        w2bd_sb = []
        for s in range(SLOTS):
            tl = P.tile([128, 64], BF16, tag=f"w2bd{s}")
            nc.sync.dma_start(out=tl[:], in_=t["w2bd"].ap()[s])
            w2bd_sb.append(tl)
        nd_sb = []
        for jt in range(NT):
            tl = P.tile([128, N], BF16, tag=f"nd{jt}")
            nc.sync.dma_start(out=tl[:], in_=t["notdiag"].ap()[jt])
            nd_sb.append(tl)

        # ---------- x -> x^T ----------
        xT = P.tile([D, N], F32, tag="xT")
        for it in range(NT):
            xin = XW.tile([128, D], F32)
            nc.sync.dma_start(out=xin[:], in_=t["nodes"].ap()[bass.ts(it, 128), :])
            px = PT.tile([128, 128], F32, tag="pt")
            nc.tensor.transpose(px[:], xin[:], idf[:])
            nc.vector.tensor_copy(out=xT[:, bass.ts(it, 128)], in_=px[:])

        # ---------- tiny MLPs (transposed; h on partitions) ----------
        def leaky_from_psum(psum, bias, tag):
            z = EW.tile([H, N], F32, tag=tag + "_z")
            nc.scalar.activation(out=z[:], in_=psum[:H, :], func=AF.Identity,
                                 bias=bias[:], scale=1.0)
            h1 = P.tile([H, N], F32, tag=tag)
            nc.vector.scalar_tensor_tensor(out=h1[:], in0=z[:], scalar=0.2,
                                           in1=z[:], op0=OP.mult, op1=OP.max)
            return h1

        pm = PM.tile([128, N], F32, tag="mm")
        nc.tensor.matmul(pm[:H, :], w1s[:], xT[:], start=True, stop=True)
        h1T_s = leaky_from_psum(pm, b1s, "h1T_s")

        pm = PM.tile([128, N], F32, tag="mm")
        nc.tensor.matmul(pm[:H, :], w1n[:], xT[:], start=True, stop=True)
        h1T_n = leaky_from_psum(pm, b1n, "h1T_n")

        pm = PM.tile([128, N], F32, tag="mm")
        nc.tensor.matmul(pm[:H, :], w2s[:], h1T_s[:], start=True, stop=True)
        eT_s = P.tile([H, N], F32, tag="eT_s")
        nc.scalar.activation(out=eT_s[:], in_=pm[:H, :], func=AF.Identity,
                             bias=b2sc[:], scale=1.0)

        pm = PM.tile([128, N], F32, tag="mm")
        nc.tensor.matmul(pm[:H, :], w2n[:], h1T_n[:], start=True, stop=True)
        eT_n = P.tile([H, N], F32, tag="eT_n")
        nc.scalar.activation(out=eT_n[:], in_=pm[:H, :], func=AF.Identity,
                             bias=b2nc[:], scale=1.0)

        # U^T -> U2 [128 (g,h), 256 pairs]
        pm = PM.tile([128, N], F32, tag="mm")
        nc.tensor.matmul(pm[:H, :], w1cs[:], eT_s[:], start=True, stop=True)
        U2 = P.tile([128, NPAIR], F32, tag="U2")
        psplit = pm[:H, :].rearrange("p (i g) -> p i g", g=2)
        nc.vector.tensor_copy(out=U2[:H, :], in_=psplit[:, :, 0])
        nc.vector.tensor_copy(out=U2[H:, :], in_=psplit[:, :, 1])

        # V^T (f32, with b1_c) and Vrep (bf16, both partition halves)
        pm = PM.tile([128, N], F32, tag="mm")
        nc.tensor.matmul(pm[:H, :], w1cn[:], eT_n[:], start=True, stop=True)
        VT = P.tile([H, N], F32, tag="VT")
        nc.scalar.activation(out=VT[:], in_=pm[:H, :], func=AF.Identity,
                             bias=b1c[:], scale=1.0)
        Vrep = P.tile([128, N], BF16, tag="Vrep")
        nc.vector.tensor_copy(out=Vrep[:H, :], in_=VT[:])
        nc.vector.tensor_copy(out=Vrep[H:, :], in_=VT[:])

        # exp(0.2 * sV) row -> scatter to [128, NT] per-partition scalars
        pm = PM.tile([128, N], F32, tag="mm")
        nc.tensor.matmul(pm[:1, :], w2c[:], VT[:], start=True, stop=True)
        sv_row = SM.tile([1, N], F32, tag="sv_row")
        nc.scalar.activation(out=sv_row[:], in_=pm[:1, :], func=AF.Exp, scale=0.2)
        nc.sync.dma_start(out=t["scr_sv"].ap().rearrange("(o f) -> o f", o=1),
                          in_=sv_row[:])
        esv = P.tile([128, NT], F32, tag="esv")
        nc.sync.dma_start(out=esv[:], in_=t["scr_sv"].ap().rearrange("(t p) -> p t", p=128))

        # ---------- mask tiles: edges * notdiag * exp(0.2 sV_j) ----------
        masks = []
        for jt in range(NT):
            esb = EW.tile([128, N], I32, tag="edges_in")
            nc.sync.dma_start(out=esb[:], in_=t["edges"].ap()[bass.ts(jt, 128), :])
            mj = P.tile([128, N], BF16, tag=f"mask{jt}")
            nc.vector.scalar_tensor_tensor(out=mj[:], in0=esb[:], scalar=esv[:, jt:jt + 1],
                                           in1=nd_sb[jt][:], op0=OP.mult, op1=OP.mult)
            masks.append(mj)

        # ---------- self_e / nb_e in [row, H] layout ----------
        selfe, nbe = [], []
        for it in range(NT):
            pa = PT.tile([128, H], F32, tag="pt", name="pa", padded_shape=[128, 128])
            nc.tensor.matmul(pa[:], h1T_s[:, bass.ts(it, 128)], w2s[:], start=True, stop=False)
            nc.tensor.matmul(pa[:], onesr[:], b2sr[:], start=False, stop=True)
            se = P.tile([128, H], F32, tag=f"selfe{it}")
            nc.scalar.copy(out=se[:], in_=pa[:])
            selfe.append(se)
        for jt in range(NT):
            pa = PT.tile([128, H], F32, tag="pt", name="pa", padded_shape=[128, 128])
            nc.tensor.matmul(pa[:], h1T_n[:, bass.ts(jt, 128)], w2n[:], start=True, stop=False)
            nc.tensor.matmul(pa[:], onesr[:], b2nr[:], start=False, stop=True)
            ne = P.tile([128, H], BF16, tag=f"nbe{jt}")
            nc.scalar.copy(out=ne[:], in_=pa[:])
            nbe.append(ne)

        # ---------- main pass: scores -> exp -> E^T ----------
        ET = [P.tile([128, N], BF16, tag=f"ET{jt}", name=f"ET{jt}") for jt in range(NT)]
        pat = BUILD_PATTERN
        for it in range(NT):
            ps = PR.tile([128, N], F32, tag="psumR")
            for c in range(2):
                for s in range(SLOTS):
                    p = 64 * it + 32 * c + s
                    rl = RL.tile([128, N], BF16, tag="relu")
                    eng = pat[p % len(pat)]
                    if eng == "v":
                        nc.vector.tensor_scalar(out=rl[:], in0=Vrep[:],
                                                scalar1=U2[:, p:p + 1], scalar2=0.0,
                                                op0=OP.add, op1=OP.max)
                    elif eng == "a":
                        nc.scalar.activation(out=rl[:], in_=Vrep[:], func=AF.Relu,
                                             bias=U2[:, p:p + 1], scale=1.0)
                    else:
                        nc.gpsimd.tensor_scalar(out=rl[:], in0=Vrep[:],
                                                scalar1=U2[:, p:p + 1], scalar2=0.0,
                                                op0=OP.add, op1=OP.max)
                    nc.tensor.matmul(ps[bass.ts(c, 64), :], w2bd_sb[s][:], rl[:],
                                     start=(s == 0), stop=(s == SLOTS - 1))
            X = XE.tile([128, N], BF16, tag="X")
            nc.scalar.activation(out=X[:], in_=ps[:], func=AF.Exp)
            for jt in range(NT):
                px = PT.tile([128, 128], BF16, tag="pt")
                nc.tensor.transpose(px[:], X[:, bass.ts(jt, 128)], idb[:])
                nc.vector.tensor_mul(out=ET[jt][:, bass.ts(it, 128)], in0=px[:],
                                     in1=masks[jt][:, bass.ts(it, 128)])

        # ---------- denom / recip / gate ----------
        pd = PM.tile([128, N], F32, tag="mm")
        for jt in range(NT):
            nc.tensor.matmul(pd[:1, :], onesc[:], ET[jt][:], start=(jt == 0),
                             stop=(jt == NT - 1))
        den_row = SM.tile([1, N], F32, tag="den_row")
        nc.vector.tensor_copy(out=den_row[:], in_=pd[:1, :])
        nc.sync.dma_start(out=t["scr_den"].ap().rearrange("(o f) -> o f", o=1),
                          in_=den_row[:])
        den = SM.tile([128, NT], F32, tag="den")
        nc.sync.dma_start(out=den[:], in_=t["scr_den"].ap().rearrange("(t p) -> p t", p=128))
        gate = SM.tile([128, NT], F32, tag="gate")
        nc.vector.tensor_single_scalar(out=gate[:], in_=den[:], scalar=1e-6, op=OP.is_gt)
        dsafe = SM.tile([128, NT], F32, tag="dsafe")
        nc.vector.tensor_scalar_max(out=dsafe[:], in0=den[:], scalar1=1e-30)
        recip = SM.tile([128, NT], F32, tag="recip")
        nc.vector.reciprocal(out=recip[:], in_=dsafe[:])
        recipg = SM.tile([128, NT], F32, tag="recipg")
        nc.vector.tensor_mul(out=recipg[:], in0=recip[:], in1=gate[:])

        # ---------- aggregate + output ----------
        for it in range(NT):
            pa = PT.tile([128, H], F32, tag="pt", name="pa", padded_shape=[128, 128])
            for jt in range(NT):
                nc.tensor.matmul(pa[:], ET[jt][:, bass.ts(it, 128)], nbe[jt][:],
                                 start=(jt == 0), stop=(jt == NT - 1))
            sg = SM.tile([128, H], F32, tag="sg")
            nc.vector.tensor_scalar_mul(out=sg[:], in0=selfe[it][:],
                                        scalar1=gate[:, it:it + 1])
            ot = SM.tile([128, H], F32, tag="ot")
            nc.vector.scalar_tensor_tensor(out=ot[:], in0=pa[:],
                                           scalar=recipg[:, it:it + 1], in1=sg[:],
                                           op0=OP.mult, op1=OP.add)
            nc.sync.dma_start(out=t["out"].ap()[bass.ts(it, 128), :], in_=ot[:])


def _host_constants(inputs):
    f32 = np.float32
    bf = ml_dtypes.bfloat16
    H_ = H
    w2 = np.asarray(inputs["comb_w2"], f32)            # [H, 1]
    w2bd = np.zeros((SLOTS, 128, 64), f32)
    for s in range(SLOTS):
        w2bd[s, 0:H_, 2 * s] = 0.8 * w2[:, 0]
        w2bd[s, H_:128, 2 * s + 1] = 0.8 * w2[:, 0]
    nd = np.ones((NT, 128, N), f32)
    for jt in range(NT):
        for p in range(128):
            nd[jt, p, jt * 128 + p] = 0.0
    consts = {
        "w1_self": np.asarray(inputs["self_w1"], f32),
        "w2_self": np.asarray(inputs["self_w2"], f32),
        "w1_nb": np.asarray(inputs["nb_w1"], f32),
        "w2_nb": np.asarray(inputs["nb_w2"], f32),
        "w1_cs": np.ascontiguousarray(np.asarray(inputs["comb_w1"], f32)[:H_]),
        "w1_cn": np.ascontiguousarray(np.asarray(inputs["comb_w1"], f32)[H_:]),
        "w2_c": w2,
        "b1_self": np.asarray(inputs["self_b1"], f32).reshape(H_, 1),
        "b1_nb": np.asarray(inputs["nb_b1"], f32).reshape(H_, 1),
        "b2_self_c": np.asarray(inputs["self_b2"], f32).reshape(H_, 1),
        "b2_nb_c": np.asarray(inputs["nb_b2"], f32).reshape(H_, 1),
        "b2_self_r": np.asarray(inputs["self_b2"], f32).reshape(1, H_),
        "b2_nb_r": np.asarray(inputs["nb_b2"], f32).reshape(1, H_),
        "b1_c": np.asarray(inputs["comb_b1"], f32).reshape(H_, 1),
        "id_f32": np.eye(128, dtype=f32),
        "id_bf16": np.eye(128, dtype=f32).astype(bf),
        "w2bd": w2bd.astype(bf),
        "ones_r": np.ones((1, 128), f32),
        "ones_c": np.ones((128, 1), f32).astype(bf),
        "notdiag": nd.astype(bf),
    }
    return consts


def kernel(**inputs):
    if "nc" not in _CACHE:
        _CACHE["nc"] = _build_module()
    nc = _CACHE["nc"]

    consts = _host_constants(inputs)
    nodes = np.asarray(inputs["nodes"], np.float32).reshape(B, N, D)
    edges = np.ascontiguousarray(np.asarray(inputs["edges"], np.int32))

    in_maps = []
    for c in range(NCORES):
        m = dict(consts)
        m["nodes"] = np.ascontiguousarray(nodes[c])
        m["edges"] = edges[c]
        in_maps.append(m)

    res = run_bass_kernel_spmd(nc, in_maps, core_ids=list(range(NCORES)))
    return np.stack([res.results[c]["out"] for c in range(NCORES)]).astype(np.float32)


# revision 22
# speedup vs baseline: 393.1039x; 393.1039x over previous
"""GAT message-passing kernel for Trainium2 (8 NeuronCores, data-parallel over batch).

Math (per batch element b, derived from the reference nn.Module):
    x        = nodes.reshape(N, D)
    self_e   = mlp2(x, self_*)            # [N, H]
    nb_e     = mlp2(x, nb_*)              # [N, H]
    U        = self_e @ comb_w1[:H]       # [N, H]   (i side)
    V        = nb_e @ comb_w1[H:] + comb_b1  # [N, H] (j side)
    scores(i,j) = leaky(U_i + V_j) @ comb_w2 + comb_b2
                = 0.8 * relu(U_i + V_j) @ w2 + 0.2*(sU_i + sV_j) + const_i
    softmax over j is invariant to per-i constants, so only
      s'(i,j) = 0.8 * relu(U_i + V_j) @ w2 + 0.2 * sV_j  matters.
    E^T[j,i] = edges[j,i] * (j != i) * exp(0.2*sV_j) * exp(0.8*relu(U_i+V_j)@w2)
    denom[i] = sum_j E^T[j,i];  gate = denom > eps;  recip = gate/denom
    out[i]   = gate * (recip * (E^T)^T @ nb_e + self_e)

Device mapping (one core per batch element):
  - Transposed "h-on-partitions" layout: partitions = (g, h) with g = i parity,
    so each DVE/ACT op builds relu(V + U_i) for TWO i's at once: [128, 512].
  - PE reduces over (g,h) with a slotted block-diagonal w2 lhsT (M=32), 16
    accumulating matmuls per 32-partition column group -> scores [128i, 512j]
    stacked in one PSUM bank.
  - ACT applies exp straight out of PSUM (bf16 out), PE transposes 128x128
    chunks, DVE multiplies by the (mask * exp(0.2 sV)) tiles -> E^T.
  - PE: ones-matmul row-reduce for denom, E^T @ nb_e for aggregation.
"""

import os
import sys

sys.path.insert(0, "/opt/trn_rl_repo")

import numpy as np
import ml_dtypes

import concourse.bass as bass
import concourse.bacc as bacc
import concourse.tile as tile
from concourse import mybir, bass2jax
from concourse.bass_utils import run_bass_kernel_spmd

B, N, H, D = 8, 512, 64, 128
NCORES = 8
NT = N // 128          # 4 i/j tiles of 128
NPAIR = N // 2         # 256 i-pairs
SLOTS = 32             # i-pairs per 64-partition column group
F32 = mybir.dt.float32
BF16 = mybir.dt.bfloat16
I32 = mybir.dt.int32

# Build-engine schedule for the 256 relu'd tiles: 'v' = VectorE, 'a' = ScalarE,
# 'p' = gpsimd/Pool. Tuned from profiles.
BUILD_PATTERN = os.environ.get("GAT_BUILD_PATTERN", "vvpvavpvvpvavpvv")

_CACHE = {}


def _build_module():
    nc = bacc.Bacc("TRN2", target_bir_lowering=False, debug=False, num_devices=NCORES)

    # ---- per-core data ----
    nodes = nc.dram_tensor("nodes", [N, D], F32, kind="ExternalInput")
    edges = nc.dram_tensor("edges", [N, N], mybir.dt.uint8, kind="ExternalInput")
    # ---- parameters / host-prepared constants (same on all cores) ----
    w1_self = nc.dram_tensor("w1_self", [D, H], F32, kind="ExternalInput")
    w2_self = nc.dram_tensor("w2_self", [H, H], F32, kind="ExternalInput")
    w1_nb = nc.dram_tensor("w1_nb", [D, H], F32, kind="ExternalInput")
    w2_nb = nc.dram_tensor("w2_nb", [H, H], F32, kind="ExternalInput")
    w1_cs = nc.dram_tensor("w1_cs", [H, H], F32, kind="ExternalInput")
    w1_cn = nc.dram_tensor("w1_cn", [H, H], F32, kind="ExternalInput")
    w2_c = nc.dram_tensor("w2_c", [H, 1], F32, kind="ExternalInput")
    bvec = nc.dram_tensor("bvec", [H, 5], F32, kind="ExternalInput")
    rowpack = nc.dram_tensor("rowpack", [1, 256], F32, kind="ExternalInput")
    id_f32 = nc.dram_tensor("id_f32", [128, 128], F32, kind="ExternalInput")
    id_bf16 = nc.dram_tensor("id_bf16", [128, 128], BF16, kind="ExternalInput")
    w2bdpack = nc.dram_tensor("w2bdpack", [128, 2], BF16, kind="ExternalInput")
    inveye = nc.dram_tensor("inveye", [128, 128], BF16, kind="ExternalInput")

    out = nc.dram_tensor("out", [N, H], F32, kind="ExternalOutput")

    scr_den = nc.dram_tensor("scr_den", [N], F32)
    scr_sv = nc.dram_tensor("scr_sv", [N], F32)

    with tile.TileContext(nc) as tc:
        _emit(nc, tc, locals())
    nc.compile()
    return nc


def _emit(nc, tc, t):
    AF = mybir.ActivationFunctionType
    OP = mybir.AluOpType

    with (
        tc.tile_pool(name="persist", bufs=1) as P,
        tc.tile_pool(name="xwork", bufs=2) as XW,
        tc.tile_pool(name="ework", bufs=2) as EW,
        tc.tile_pool(name="relu", bufs=14) as RL,
        tc.tile_pool(name="xexp", bufs=3) as XE,
        tc.tile_pool(name="small", bufs=4) as SM,
        tc.tile_pool(name="psumR", bufs=2, space="PSUM") as PR,
        tc.tile_pool(name="psumT", bufs=3, space="PSUM") as PT,
        tc.tile_pool(name="psumM", bufs=2, space="PSUM") as PM,
        tc.tile_pool(name="psumA", bufs=1, space="PSUM") as PA,
    ):
        # ---------- load constants ----------
        def load(name, shape, dtype, eng=None):
            tl = P.tile(shape, dtype, tag=name)
            (eng or nc.sync).dma_start(out=tl[:], in_=t[name].ap())
            return tl

        xins = []
        for it in range(NT):
            xin = XW.tile([128, D], F32, name="xin", tag="xin")
            nc.sync.dma_start(out=xin[:], in_=t["nodes"].ap()[bass.ts(it, 128), :])
            xins.append(xin)
        w1s = load("w1_self", [D, H], F32, eng=nc.scalar)
        w1n = load("w1_nb", [D, H], F32, eng=nc.scalar)
        w2s = load("w2_self", [H, H], F32, eng=nc.scalar)
        w2n = load("w2_nb", [H, H], F32, eng=nc.scalar)
        w1cs = load("w1_cs", [H, H], F32, eng=nc.scalar)
        w1cn = load("w1_cn", [H, H], F32, eng=nc.scalar)
        w2c = load("w2_c", [H, 1], F32, eng=nc.scalar)
        bvec = load("bvec", [H, 5], F32, eng=nc.sync)
        b1s, b1n = bvec[:, 0:1], bvec[:, 1:2]
        b2sc, b2nc, b1c = bvec[:, 2:3], bvec[:, 3:4], bvec[:, 4:5]
        rowp = load("rowpack", [1, 256], F32, eng=nc.sync)
        onesr, b2sr, b2nr = rowp[:, 0:128], rowp[:, 128:192], rowp[:, 192:256]
        idf = P.tile([128, 128], F32, tag="id_f32")
        nc.gpsimd.dma_start(out=idf[:], in_=t["id_f32"].ap())
        idb = P.tile([128, 128], BF16, tag="id_bf16")
        nc.gpsimd.dma_start(out=idb[:], in_=t["id_bf16"].ap())
        onesc = P.tile([128, 1], BF16, tag="onesc")
        nc.gpsimd.memset(onesc[:], 1.0)
        ive = P.tile([128, 128], BF16, tag="ive")
        nc.gpsimd.dma_start(out=ive[:], in_=t["inveye"].ap())
        w2bd_all = P.tile([128, 128], BF16, tag="w2bd_all")
        nc.gpsimd.memset(w2bd_all[:], 0.0)
        nc.gpsimd.dma_start(out=w2bd_all[:, 62:64], in_=t["w2bdpack"].ap())
        w2bd_sb = [w2bd_all[:, 62 - 2 * s:126 - 2 * s] for s in range(SLOTS)]

        # ---------- x -> x^T (bf16 for the small MLP matmuls) ----------
        xT = P.tile([D, N], F32, tag="xT")
        for it in range(NT):
            px = PT.tile([128, 128], F32, tag="pt", name="px", padded_shape=[128, 128])
            nc.tensor.transpose(px[:], xins[it][:], idf[:])
            nc.vector.tensor_copy(out=xT[:, bass.ts(it, 128)], in_=px[:])

        # ---------- tiny MLPs (transposed; h on partitions) ----------
        def leaky_from_psum(psum, bias, tag):
            z = EW.tile([H, N], F32, tag="lk_z")
            nc.scalar.activation(out=z[:], in_=psum[:H, :], func=AF.Identity,
                                 bias=bias, scale=1.0)
            h1 = P.tile([H, N], F32, tag=tag, name=tag)
            nc.vector.scalar_tensor_tensor(out=h1[:], in0=z[:], scalar=0.2,
                                           in1=z[:], op0=OP.mult, op1=OP.max)
            return h1

        pm = PM.tile([128, N], F32, tag="mm")
        nc.tensor.matmul(pm[:H, :], w1n[:], xT[:], start=True, stop=True)
        h1T_n = leaky_from_psum(pm, b1n, "h1T_n")

        pm = PM.tile([128, N], F32, tag="mm")
        nc.tensor.matmul(pm[:H, :], w2n[:], h1T_n[:], start=True, stop=True)
        eT_n = P.tile([H, N], F32, tag="eT_n")
        nc.scalar.activation(out=eT_n[:], in_=pm[:H, :], func=AF.Identity,
                             bias=b2nc, scale=1.0)

        # V^T (f32, with b1_c) and Vrep (bf16, both partition halves)
        pm = PM.tile([128, N], F32, tag="mm")
        nc.tensor.matmul(pm[:H, :], w1cn[:], eT_n[:], start=True, stop=True)
        VT = P.tile([H, N], F32, tag="VT")
        nc.scalar.activation(out=VT[:], in_=pm[:H, :], func=AF.Identity,
                             bias=b1c, scale=1.0)
        Vrep = P.tile([128, N], BF16, tag="Vrep")
        nc.vector.tensor_copy(out=Vrep[:H, :], in_=VT[:])
        nc.vector.tensor_copy(out=Vrep[H:, :], in_=VT[:])

        # self chain, chunked by 128 i-columns so U2's early columns land
        # early (first builds only need U2[:, 0:64])
        h1T_s = P.tile([H, N], F32, tag="h1T_s")
        eT_s = P.tile([H, N], F32, tag="eT_s")
        U2 = P.tile([128, NPAIR], F32, tag="U2")
        for it in range(NT):
            cs = bass.ts(it, 128)
            pm = PT.tile([128, 128], F32, tag="pt", name="pmc")
            nc.tensor.matmul(pm[:H, :], w1s[:], xT[:, cs], start=True, stop=True)
            zc = EW.tile([H, 128], F32, tag="lk_zc", name="zc")
            nc.scalar.activation(out=zc[:], in_=pm[:H, :], func=AF.Identity,
                                 bias=b1s, scale=1.0)
            nc.vector.scalar_tensor_tensor(out=h1T_s[:, cs], in0=zc[:], scalar=0.2,
                                           in1=zc[:], op0=OP.mult, op1=OP.max)
            pm = PT.tile([128, 128], F32, tag="pt", name="pmc")
            nc.tensor.matmul(pm[:H, :], w2s[:], h1T_s[:, cs], start=True, stop=True)
            nc.scalar.activation(out=eT_s[:, cs], in_=pm[:H, :], func=AF.Identity,
                                 bias=b2sc, scale=1.0)
            pm = PT.tile([128, 128], F32, tag="pt", name="pmc")
            nc.tensor.matmul(pm[:H, :], w1cs[:], eT_s[:, cs], start=True, stop=True)
            psplit = pm[:H, :].rearrange("p (i g) -> p i g", g=2)
            nc.vector.tensor_copy(out=U2[:H, bass.ts(it, 64)], in_=psplit[:, :, 0])
            nc.vector.tensor_copy(out=U2[H:, bass.ts(it, 64)], in_=psplit[:, :, 1])

        # exp(0.2 * sV) row -> scatter to [128, NT] per-partition scalars
        pm = PM.tile([128, N], F32, tag="mm")
        nc.tensor.matmul(pm[:1, :], w2c[:], VT[:], start=True, stop=True)
        sv_row = SM.tile([1, N], F32, tag="sv_row")
        nc.scalar.activation(out=sv_row[:], in_=pm[:1, :], func=AF.Exp, scale=0.2)
        nc.sync.dma_start(out=t["scr_sv"].ap().rearrange("(o f) -> o f", o=1),
                          in_=sv_row[:])
        esv = P.tile([128, NT], F32, tag="esv")
        nc.sync.dma_start(out=esv[:], in_=t["scr_sv"].ap().rearrange("(t p) -> p t", p=128))

        # ---------- mask tiles: edges * notdiag * exp(0.2 sV_j) ----------
        masks = []
        for jt in range(NT):
            esb = EW.tile([128, N], mybir.dt.uint8, tag="edges_in")
            nc.gpsimd.dma_start(out=esb[:], in_=t["edges"].ap()[bass.ts(jt, 128), :])
            mj = P.tile([128, N], BF16, tag=f"mask{jt}", name=f"mask{jt}")
            nc.vector.tensor_scalar_mul(out=mj[:], in0=esb[:], scalar1=esv[:, jt:jt + 1])
            nc.vector.tensor_mul(out=mj[:, bass.ts(jt, 128)], in0=mj[:, bass.ts(jt, 128)],
                                 in1=ive[:])
            masks.append(mj)

        # ---------- self_e / nb_e in [row, H] layout ----------
        selfe, nbe = [], []
        for it in range(NT):
            pa = PT.tile([128, H], F32, tag="pt", name="pa", padded_shape=[128, 128])
            nc.tensor.matmul(pa[:], h1T_s[:, bass.ts(it, 128)], w2s[:], start=True, stop=False)
            nc.tensor.matmul(pa[:], onesr, b2sr, start=False, stop=True)
            se = P.tile([128, H], F32, tag=f"selfe{it}")
            nc.scalar.copy(out=se[:], in_=pa[:])
            selfe.append(se)
        for jt in range(NT):
            pa = PT.tile([128, H], F32, tag="pt", name="pa", padded_shape=[128, 128])
            nc.tensor.matmul(pa[:], h1T_n[:, bass.ts(jt, 128)], w2n[:], start=True, stop=False)
            nc.tensor.matmul(pa[:], onesr, b2nr, start=False, stop=True)
            ne = P.tile([128, H], BF16, tag=f"nbe{jt}")
            nc.scalar.copy(out=ne[:], in_=pa[:])
            nbe.append(ne)

        # ---------- main pass: scores -> exp -> E^T -> denom/agg ----------
        ET = [P.tile([128, N], BF16, tag=f"ET{jt}", name=f"ET{jt}") for jt in range(NT)]
        pat = BUILD_PATTERN
        pd = PM.tile([128, N], F32, tag="mm")
        pa_all = PA.tile([128, NT, H], F32, tag="pa_all")
        for it in range(NT):
            ps = PR.tile([128, N], F32, tag="psumR")
            for c in range(2):
                for s in range(SLOTS):
                    p = 64 * it + 32 * c + s
                    rl = RL.tile([128, N], BF16, tag="relu")
                    eng = pat[p % len(pat)]
                    if eng == "v":
                        nc.vector.tensor_scalar(out=rl[:], in0=Vrep[:],
                                                scalar1=U2[:, p:p + 1], scalar2=0.0,
                                                op0=OP.add, op1=OP.max)
                    elif eng == "a":
                        nc.scalar.activation(out=rl[:], in_=Vrep[:], func=AF.Relu,
                                             bias=U2[:, p:p + 1], scale=1.0)
                    else:
                        nc.gpsimd.tensor_scalar(out=rl[:], in0=Vrep[:],
                                                scalar1=U2[:, p:p + 1], scalar2=0.0,
                                                op0=OP.add, op1=OP.max)
                    nc.tensor.matmul(ps[bass.ts(c, 64), :], w2bd_sb[s], rl[:],
                                     start=(s == 0), stop=(s == SLOTS - 1))
            X = XE.tile([128, N], BF16, tag="X")
            nc.scalar.activation(out=X[:], in_=ps[:], func=AF.Exp)
            for jt in range(NT):
                px = PT.tile([128, 128], BF16, tag="pt")
                nc.tensor.transpose(px[:], X[:, bass.ts(jt, 128)], idb[:])
                nc.vector.tensor_mul(out=ET[jt][:, bass.ts(it, 128)], in0=px[:],
                                     in1=masks[jt][:, bass.ts(it, 128)])
            # denom partial: accumulate sum_j over this it's column block
            for jt in range(NT):
                nc.tensor.matmul(pd[:1, bass.ts(it, 128)], onesc[:],
                                 ET[jt][:, bass.ts(it, 128)],
                                 start=(jt == 0), stop=(jt == NT - 1))
            # aggregation for this i-tile
            for jt in range(NT):
                nc.tensor.matmul(pa_all[:, it, :], ET[jt][:, bass.ts(it, 128)], nbe[jt][:],
                                 start=(jt == 0), stop=(jt == NT - 1))
            # denom roundtrip for this i-tile: [1,128] row -> [128,1] column
            den_row = SM.tile([1, 128], F32, tag="den_row")
            nc.vector.tensor_copy(out=den_row[:], in_=pd[:1, bass.ts(it, 128)])
            nc.sync.dma_start(
                out=t["scr_den"].ap()[bass.ts(it, 128)].rearrange("(o f) -> o f", o=1),
                in_=den_row[:])
            den = SM.tile([128, 1], F32, tag="den", name="den")
            nc.sync.dma_start(
                out=den[:], in_=t["scr_den"].ap()[bass.ts(it, 128)].rearrange("(p o) -> p o", o=1))
            gate = SM.tile([128, 1], F32, tag="gate", name="gate")
            nc.vector.tensor_single_scalar(out=gate[:], in_=den[:], scalar=1e-6, op=OP.is_gt)
            dsafe = SM.tile([128, 1], F32, tag="dsafe", name="dsafe")
            nc.vector.tensor_scalar_max(out=dsafe[:], in0=den[:], scalar1=1e-30)
            recipg = SM.tile([128, 1], F32, tag="recipg", name="recipg")
            nc.vector.reciprocal(out=recipg[:], in_=dsafe[:])
            # output assembly for this i-tile
            sg = SM.tile([128, H], F32, tag="sg")
            nc.vector.tensor_scalar_mul(out=sg[:], in0=selfe[it][:], scalar1=gate[:])
            nc.vector.tensor_mul(out=recipg[:], in0=recipg[:], in1=gate[:])
            ot = SM.tile([128, H], F32, tag="ot")
            nc.vector.scalar_tensor_tensor(out=ot[:], in0=pa_all[:, it, :],
                                           scalar=recipg[:], in1=sg[:],
                                           op0=OP.mult, op1=OP.add)
            nc.sync.dma_start(out=t["out"].ap()[bass.ts(it, 128), :], in_=ot[:])


def _host_constants(inputs):
    f32 = np.float32
    bf = ml_dtypes.bfloat16
    H_ = H
    w2 = np.asarray(inputs["comb_w2"], f32)            # [H, 1]
    w2bdpack = np.zeros((128, 2), f32)
    w2bdpack[0:H_, 0] = 0.8 * w2[:, 0]
    w2bdpack[H_:128, 1] = 0.8 * w2[:, 0]
    ive = (1.0 - np.eye(128)).astype(f32)
    consts = {
        "w1_self": np.asarray(inputs["self_w1"], f32),
        "w2_self": np.asarray(inputs["self_w2"], f32),
        "w1_nb": np.asarray(inputs["nb_w1"], f32),
        "w2_nb": np.asarray(inputs["nb_w2"], f32),
        "w1_cs": np.ascontiguousarray(np.asarray(inputs["comb_w1"], f32)[:H_]),
        "w1_cn": np.ascontiguousarray(np.asarray(inputs["comb_w1"], f32)[H_:]),
        "w2_c": w2,
        "bvec": np.stack([
            np.asarray(inputs["self_b1"], f32),
            np.asarray(inputs["nb_b1"], f32),
            np.asarray(inputs["self_b2"], f32),
            np.asarray(inputs["nb_b2"], f32),
            np.asarray(inputs["comb_b1"], f32),
        ], axis=1),
        "rowpack": np.concatenate([
            np.ones(128, f32),
            np.asarray(inputs["self_b2"], f32),
            np.asarray(inputs["nb_b2"], f32),
        ]).reshape(1, 256),
        "id_f32": np.eye(128, dtype=f32),
        "id_bf16": np.eye(128, dtype=f32).astype(bf),
        "w2bdpack": w2bdpack.astype(bf),
        "inveye": ive.astype(bf),
    }
    return consts


def _build_fast_path(nc):
    """Cache a single jitted shard_map executable so repeat kernel() calls
    skip jax re-tracing (same lowering run_bass_kernel_spmd uses under axon)."""
    import jax
    from jax.sharding import Mesh, PartitionSpec
    from jax.experimental.shard_map import shard_map

    bass2jax.install_neuronx_cc_hook()
    pname = nc.partition_id_tensor.name if nc.partition_id_tensor else None
    in_names, out_names, out_avals = [], [], []
    for alloc in nc.m.functions[0].allocations:
        if not isinstance(alloc, mybir.MemoryLocationSet):
            continue
        name = alloc.memorylocations[0].name
        if alloc.kind == "ExternalInput":
            if name != pname:
                in_names.append(name)
        elif alloc.kind == "ExternalOutput":
            out_names.append(name)
            out_avals.append(jax.core.ShapedArray(tuple(alloc.tensor_shape),
                                                  mybir.dt.np(alloc.dtype)))
    all_names = in_names + out_names + ([pname] if pname else [])

    def _body(*args):
        operands = list(args)
        if pname is not None:
            operands.append(bass2jax.partition_id_tensor())
        return tuple(bass2jax._bass_exec_p.bind(
            *operands, out_avals=tuple(out_avals), in_names=tuple(all_names),
            out_names=tuple(out_names), lowering_input_output_aliases=(),
            sim_require_finite=True, sim_require_nnan=True, nc=nc))

    devices = jax.devices()[:NCORES]
    mesh = Mesh(np.asarray(devices), ("core",))
    n_io = len(in_names) + len(out_names)
    sharded = jax.jit(
        shard_map(_body, mesh=mesh, in_specs=(PartitionSpec("core"),) * n_io,
                  out_specs=(PartitionSpec("core"),) * len(out_names),
                  check_rep=False),
        keep_unused=True,
    )
    return sharded, in_names, out_names, out_avals


def kernel(**inputs):
    first = "nc" not in _CACHE
    if first:
        _CACHE["nc"] = _build_module()
    nc = _CACHE["nc"]

    consts = _host_constants(inputs)
    nodes = np.asarray(inputs["nodes"], np.float32).reshape(B, N, D)
    edges = (np.asarray(inputs["edges"]) != 0).astype(np.uint8)

    in_maps = []
    for c in range(NCORES):
        m = dict(consts)
        m["nodes"] = np.ascontiguousarray(nodes[c])
        m["edges"] = edges[c]
        in_maps.append(m)

    if first:
        res = run_bass_kernel_spmd(nc, in_maps, core_ids=list(range(NCORES)))
        _CACHE["fast"] = _build_fast_path(nc)
        return np.stack([res.results[c]["out"] for c in range(NCORES)]).astype(np.float32)

    import jax
    sharded, in_names, out_names, out_avals = _CACHE["fast"]
    ckey = hash(tuple((k, v.tobytes()) for k, v in sorted(consts.items())))
    if _CACHE.get("ckey") != ckey:
        _CACHE["cdev"] = {
            n: jax.device_put(np.concatenate([np.asarray(in_maps[c][n])
                                              for c in range(NCORES)], axis=0))
            for n in in_names if n not in ("nodes", "edges")
        }
        _CACHE["zdev"] = [jax.device_put(np.zeros((NCORES * a.shape[0], *a.shape[1:]),
                                                  a.dtype)) for a in out_avals]
        _CACHE["ckey"] = ckey
    cdev = _CACHE["cdev"]
    concat_in = [cdev[n] if n in cdev else
                 np.concatenate([np.asarray(in_maps[c][n]) for c in range(NCORES)], axis=0)
                 for n in in_names]
    outs = sharded(*concat_in, *_CACHE["zdev"])
    i = out_names.index("out")
    return np.asarray(outs[i]).reshape(NCORES, N, H).astype(np.float32)
